# revision 24
# baseline (speedup 1.0000x reference)
"""Trainium2 Bass kernel for the vq_codebook autoencoder (nn_AE_control_54546084659681).

Data-parallel across 8 NeuronCores: each core processes 128 of the 1024 batch
elements; all weights are replicated. No collectives (forward only).

Encoder convs run as split-bf16 matmuls (hi/lo decomposition, 3 products per
tap-pair, f32 PSUM accumulate - ~1e-5 relative accuracy, protecting the VQ
argmin indices, at 1 cycle/row). The VQ argmax scores are computed in plain
f32 (exact). Softmax/q and the decoder run in f32r (1 cycle/row), and the
final 512x16384 FC runs in bf16. The element loop is software-pipelined:
element e's VQ + decoder stages are emitted between element e+1's encoder
stages so the in-order PE stream always has ready matmuls.

Self-contained: hardcodes all shapes; only needs /opt/trn_rl_repo on the path.
"""
import os
import sys

sys.path.insert(0, "/opt/trn_rl_repo")

import numpy as np

T = 512
D = 64
KW = 9          # conv kernel width
B = 1024
NCORES = 8
EPC = B // NCORES          # elements per core = 128
NUM_M = 64
NUM_N = 32
SCALE = 1.0
N_ELEM = int(os.environ.get("BASS_N_ELEM", str(EPC)))  # loop count (debug aid)

_COMPILED = None


def _build():
    from concourse import bacc, tile, mybir

    F32 = mybir.dt.float32
    F32R = mybir.dt.float32r
    BF16 = mybir.dt.bfloat16
    U32 = mybir.dt.uint32
    I32 = mybir.dt.int32
    AF = mybir.ActivationFunctionType
    SUB = mybir.AluOpType.subtract

    nc = bacc.Bacc("TRN2", target_bir_lowering=False, debug=False,
                   num_devices=NCORES)

    # ---------------- DRAM parameters -----------------------------------
    def din(name, shape):
        return nc.dram_tensor(name, list(shape), F32, kind="ExternalInput").ap()

    xcol_d = din("xcol", (EPC, KW, T))            # im2col'd padded input
    w0c1_d = din("w0c1", (KW, 128))               # [conv1 | shortcut] lhsT
    b1e0_d = din("b1e0", (D, 1))
    w0c2_d = din("w0c2", (5, 128, D))             # enc0 conv2 tap-pairs
    b2e0_d = din("b2e0", (D, 1))
    bse0_d = din("bse0", (D, 1))
    wenc_d = din("wenc", (3, 2, 5, 128, D))       # enc blocks tap-pairs
    benc_d = din("benc", (3, 2, D, 1))
    wdec_d = din("wdec", (3, 2, 5, 128, D))       # dec blocks, stream-stacked
    bdec_d = din("bdec", (3, 2, D, 1))            # stacked (s|n) biases
    fcwT_d = din("fcwT", (T * D // 2, 512))       # fc1_w transposed
    fcb_d = din("fcb", (512,))
    means_s_d = din("means_s", (D // 2, NUM_M))
    meansT_s_d = din("meansT_s", (NUM_M, D // 2))
    means_n_d = din("means_n", (D // 2, NUM_N))
    meansT_n_d = din("meansT_n", (NUM_N, D // 2))

    s_hat_d = nc.dram_tensor("s_hat", [EPC, 512], F32, kind="ExternalOutput").ap()
    n_hat_d = nc.dram_tensor("n_hat", [EPC, 512], F32, kind="ExternalOutput").ap()
    idx_s_d = nc.dram_tensor("idx_s", [EPC, T], I32, kind="ExternalOutput").ap()
    idx_n_d = nc.dram_tensor("idx_n", [EPC, T], I32, kind="ExternalOutput").ap()

    id64_d = nc.inline_tensor(np.eye(D, dtype=np.float32), name="id64")
    id128_d = nc.inline_tensor(np.eye(128, dtype=np.float32), name="id128")

    with tile.TileContext(nc) as tc:
        with (
            tc.tile_pool(name="wpool", bufs=1) as wpool,
            tc.tile_pool(name="zpool", bufs=1) as zpool,
            tc.tile_pool(name="iopool", bufs=1) as iopool,
            tc.tile_pool(name="psum", bufs=1, space="PSUM") as pp,
        ):
            def wtile(shape, dtype, tag):
                return wpool.tile(list(shape), dtype, tag=tag, name=tag)

            # ------------- weights: DMA f32, build bf16 hi/lo ------------
            wstage = [wtile((128, 128), F32, f"wstage{i}") for i in range(3)]
            _ws = [0]

            def wst():
                t_ = wstage[_ws[0] % 3]
                _ws[0] += 1
                return t_

            def split_pair(shape, tag, src_ap):
                tf = wst()
                nc.sync.dma_start(tf[0:shape[0], 0:shape[1]], src_ap)
                hi = wtile(shape, BF16, tag + "h")
                lo = wtile(shape, BF16, tag + "l")
                nc.vector.tensor_copy(hi[:], tf[0:shape[0], 0:shape[1]])
                nc.vector.tensor_tensor(lo[:], tf[0:shape[0], 0:shape[1]],
                                        hi[:], op=SUB)
                return hi, lo

            w0c1h, w0c1l = split_pair((KW, 128), "w0c1", w0c1_d[:])

            w0c2h, w0c2l = [], []
            for p in range(5):
                h_, l_ = split_pair((128, D), f"w0c2{p}", w0c2_d[p])
                w0c2h.append(h_)
                w0c2l.append(l_)

            wench, wencl = {}, {}
            for i in range(3):
                for l in range(2):
                    for p in range(5):
                        h_, l_ = split_pair((128, D), f"we{i}{l}{p}",
                                            wenc_d[i, l, p])
                        wench[(i, l, p)] = h_
                        wencl[(i, l, p)] = l_

            wdecr = {}
            for i in range(3):
                for l in range(2):
                    for p in range(5):
                        tf = wst()
                        nc.sync.dma_start(tf[:, 0:D], wdec_d[i, l, p])
                        tr = wtile((128, D), F32R, f"wdr{i}{l}{p}")
                        nc.vector.tensor_copy(tr[:], tf[:, 0:D])
                        wdecr[(i, l, p)] = tr

            id64f = wtile((D, D), F32, "id64f")
            nc.sync.dma_start(id64f[:], id64_d.ap()[:])
            id64r = wtile((D, D), F32R, "id64r")
            nc.vector.tensor_copy(id64r[:], id64f[:])
            id64b = wtile((D, D), BF16, "id64b")
            nc.vector.tensor_copy(id64b[:], id64f[:])
            id128f = wtile((128, 128), F32, "id128f")
            nc.sync.dma_start(id128f[:], id128_d.ap()[:])

            def bias_tile(src_ap, n, tag):
                t_ = wtile((n, 1), F32, tag)
                nc.sync.dma_start(t_[:], src_ap)
                return t_

            b1e0 = bias_tile(b1e0_d, D, "b1e0")
            b2e0 = bias_tile(b2e0_d, D, "b2e0")
            bse0 = bias_tile(bse0_d, D, "bse0")
            b2bs = wtile((D, 1), F32, "b2bs")
            nc.vector.tensor_add(b2bs[:], b2e0[:], bse0[:])
            benc = {}
            bdec = {}
            for i in range(3):
                for l in range(2):
                    benc[(i, l)] = bias_tile(benc_d[i, l], D, f"benc{i}{l}")
                    bdec[(i, l)] = bias_tile(bdec_d[i, l], D, f"bdec{i}{l}")
            fcb = wtile((128, 4), F32, "fcb")
            nc.sync.dma_start(fcb[:], fcb_d.rearrange("(ob p) -> p ob", p=128))

            # ------------- VQ codebook prep ------------------------------
            vq = {}
            for cb, (mdim, mns_d, mnsT_d) in (
                ("s", (NUM_M, means_s_d, meansT_s_d)),
                ("n", (NUM_N, means_n_d, meansT_n_d)),
            ):
                mt = wtile((mdim, 32), F32, f"mt_{cb}")
                nc.sync.dma_start(mt[:], mnsT_d[:])
                mns = wtile((32, mdim), F32, f"mns_{cb}")
                nc.sync.dma_start(mns[:], mns_d[:])
                wg = wtile((33, mdim), F32, f"wg_{cb}")
                nc.vector.tensor_scalar_mul(wg[0:32, :], mns[:], 2.0)
                sq = wtile((mdim, 32), F32, f"sq_{cb}")
                nc.vector.tensor_mul(sq[:], mt[:], mt[:])
                m2 = wtile((mdim, 1), F32, f"m2_{cb}")
                nc.vector.reduce_sum(m2[:], sq[:], axis=mybir.AxisListType.X)
                nm2 = wtile((mdim, 1), F32, f"nm2_{cb}")
                nc.vector.tensor_scalar_mul(nm2[:], m2[:], -1.0)
                pt_ = pp.tile([1, mdim], F32, tag="psA0", name="prep_t")
                nc.tensor.transpose(pt_[:], nm2[:], id64f[0:mdim, 0:mdim])
                nc.scalar.copy(wg[32:33, :], pt_[:])
                wgr = wtile((33, mdim), F32R, f"wgr_{cb}")
                nc.vector.tensor_copy(wgr[:], wg[:])
                mTa = wtile((mdim, 33), F32, f"mTa_{cb}")
                nc.vector.tensor_copy(mTa[:, 0:32], mt[:])
                nc.vector.memset(mTa[:, 32:33], 1.0)
                mTar = wtile((mdim, 33), F32R, f"mTar_{cb}")
                nc.vector.tensor_copy(mTar[:], mTa[:])
                vq[cb] = dict(mdim=mdim, wg=wg, wgr=wgr, mTar=mTar)

            ones32f = wtile((33, 32), F32, "ones32f")
            nc.vector.memset(ones32f[0:1, :], 1.0)
            nc.vector.memset(ones32f[32:33, :], 1.0)
            ones32r = wtile((33, 32), F32R, "ones32r")
            nc.vector.tensor_copy(ones32r[0:1, :], ones32f[0:1, :])
            nc.vector.tensor_copy(ones32r[32:33, :], ones32f[32:33, :])

            # ------------- ring tiles ------------------------------------
            def ring(n, shape, dtype, tag):
                return [zpool.tile(list(shape), dtype, tag=f"{tag}{i}",
                                   name=f"{tag}{i}") for i in range(n)]

            ZW = T + 9            # 521
            x9_r = ring(3, (KW, T), F32, "x9")
            x9h_r = ring(2, (KW, T), BF16, "x9h")
            x9l_r = ring(2, (KW, T), BF16, "x9l")

            def zpair_ring(tag):
                return (ring(2, (128, ZW), BF16, tag + "h"),
                        ring(2, (128, ZW), BF16, tag + "l"))

            zA_r = zpair_ring("zA")     # enc0 conv2 input
            zB_r = zpair_ring("zB")     # enc0 out / block0 in
            z1_r = zpair_ring("z1")     # block0 out
            z2_r = zpair_ring("z2")     # block1 out
            zM_r = zpair_ring("zM")     # block mids
            ztf_r = ring(3, (D, ZW), F32, "ztf")      # enc evict staging
            h3s_r = ring(2, (33, T), F32, "h3s")
            h3n_r = ring(2, (33, T), F32, "h3n")
            h3sr_r = ring(2, (33, T), F32R, "h3sr")
            h3nr_r = ring(2, (33, T), F32R, "h3nr")
            e_s_r = ring(2, (NUM_M, T), F32R, "es")
            e_n_r = ring(2, (NUM_N, T), F32R, "en")
            qun_r = ring(1, (32, T), F32, "qun")
            rcpf_r = ring(2, (33, T), F32, "rcpf")
            rcprr_r = ring(2, (33, T), F32R, "rcprr")
            go_r = ring(1, (128, 4 * NUM_M + 4 * NUM_N), F32, "go")
            mx_r = ring(2, (128, 8), F32, "mx")
            ztmp_r = ring(3, (D, ZW), F32, "ztmp")    # q/dec evict staging
            zq_r = ring(2, (128, T + 8), F32R, "zq")  # dec conv inputs
            zdm_r = ring(2, (128, T + 8), F32R, "zdm")
            ztail_r = ring(2, (D, T), F32, "ztail")
            idx_r = {"s": ring(2, (128, 16, 4, 8), U32, "idxs"),
                     "n": ring(2, (128, 16, 4, 8), U32, "idxn")}

            ZT = [iopool.tile([128, 2, EPC, 32], BF16, tag=f"zt{tb}",
                              name=f"zt{tb}") for tb in range(4)]

            for z in ztf_r + ztmp_r:
                nc.gpsimd.memset(z[:, 0:4], 0.0)
                nc.gpsimd.memset(z[:, T + 4:ZW], 0.0)
            for h in h3s_r + h3n_r:
                nc.gpsimd.memset(h[32:33, :], 1.0)

            psA_r = [pp.tile([128, T], F32, tag=f"psA{i}", name=f"psA{i}")
                     for i in range(2)]
            psC_r = [pp.tile([D, T], F32, tag=f"psC{i}", name=f"psC{i}")
                     for i in range(4)]
            psGO_t = pp.tile([128, 4 * NUM_M + 4 * NUM_N], F32, tag="psGO",
                             name="psGO")
            psT_t = pp.tile([128, 256], F32, tag="psT", name="psT")
            psW_r = [pp.tile([128, 256], F32, tag=f"psC{i}", name=f"psW{i}")
                     for i in range(2)]
            _cnt = {}

            def nxt(name, lst):
                i = _cnt.get(name, 0)
                _cnt[name] = i + 1
                return lst[i % len(lst)]

            # ------------- stage helpers ---------------------------------
            def build_split(ps_in, bias, pair, f):
                """relu+bias evict -> f32 staging -> bf16 hi/lo doubled-shift."""
                ztf = nxt("ztf", ztf_r)
                nc.scalar.activation(ztf[:, 4:T + 4], ps_in, AF.Relu,
                                     bias=bias[:])
                zh, zl = pair[0][f % 2], pair[1][f % 2]
                nc.vector.tensor_copy(zh[0:D, 0:ZW], ztf[:, 0:ZW])
                nc.gpsimd.tensor_copy(zh[D:128, 0:ZW - 1], ztf[:, 1:ZW])
                nc.vector.tensor_tensor(zl[0:D, 0:ZW], ztf[:, 0:ZW],
                                        zh[0:D, 0:ZW], op=SUB)
                nc.gpsimd.tensor_copy(zl[D:128, 0:ZW - 1], zl[0:D, 1:ZW])

            def conv15(wh, wl, zh, zl, ps, first_start, last_stop):
                prods = ([(wh[p], zh, p) for p in range(5)]
                         + [(wl[p], zh, p) for p in range(5)]
                         + [(wh[p], zl, p) for p in range(5)])
                for k, (lhs, rhs, p) in enumerate(prods):
                    nc.tensor.matmul(
                        ps, lhs[:], rhs[:, 2 * p:2 * p + T],
                        start=(k == 0 and first_start),
                        stop=(k == 14 and last_stop),
                        skip_group_check=True)

            # --- encoder stages (element f = e+1 pipelined) ---
            def st_enc0_mm1(f):
                x9 = x9_r[f % 3]
                x9h, x9l = x9h_r[f % 2], x9l_r[f % 2]
                nc.vector.tensor_copy(x9h[:], x9[:])
                nc.vector.tensor_tensor(x9l[:], x9[:], x9h[:], op=SUB)
                psA = psA_r[f % 2]
                nc.tensor.matmul(psA[:], w0c1h[:], x9h[:], start=True,
                                 stop=False)
                nc.tensor.matmul(psA[:], w0c1l[:], x9h[:], start=False,
                                 stop=False, skip_group_check=True)
                nc.tensor.matmul(psA[:], w0c1h[:], x9l[:], start=False,
                                 stop=False, skip_group_check=True)
                build_split(psA[0:D, :], b1e0, zA_r, f)

            def st_enc0_conv2(f):
                psA = psA_r[f % 2]
                zh, zl = zA_r[0][f % 2], zA_r[1][f % 2]
                conv15(w0c2h, w0c2l, zh, zl, psA[D:128, :], False, True)
                build_split(psA[D:128, :], b2bs, zB_r, f)

            def _zin(f, i):
                src = (zB_r, z1_r, z2_r)[i]
                return src[0][f % 2], src[1][f % 2]

            def st_block_conv1(f, i):
                zh, zl = _zin(f, i)
                ps = nxt("psC", psC_r)
                conv15([wench[(i, 0, p)] for p in range(5)],
                       [wencl[(i, 0, p)] for p in range(5)],
                       zh, zl, ps[:], True, True)
                build_split(ps[:], benc[(i, 0)], zM_r, f)

            def st_block_conv2(f, i):
                zmh, zml = zM_r[0][f % 2], zM_r[1][f % 2]
                zih, zil = _zin(f, i)
                ps = nxt("psC", psC_r)
                conv15([wench[(i, 1, p)] for p in range(5)],
                       [wencl[(i, 1, p)] for p in range(5)],
                       zmh, zml, ps[:], True, False)
                nc.tensor.matmul(ps[:], id64b[:], zih[0:D, 4:T + 4],
                                 start=False, stop=False,
                                 skip_group_check=True)
                nc.tensor.matmul(ps[:], id64b[:], zil[0:D, 4:T + 4],
                                 start=False, stop=True,
                                 skip_group_check=True)
                if i < 2:
                    build_split(ps[:], benc[(i, 1)], (z1_r, z2_r)[i], f)
                else:
                    h3s, h3n = h3s_r[f % 2], h3n_r[f % 2]
                    nc.scalar.activation(h3s[0:32, :], ps[0:32, :], AF.Relu,
                                         bias=benc[(i, 1)][0:32, :])
                    nc.scalar.activation(h3n[0:32, :], ps[32:D, :], AF.Relu,
                                         bias=benc[(i, 1)][32:D, :])
                    nc.vector.tensor_copy(h3sr_r[f % 2][:], h3s[:])
                    nc.vector.tensor_copy(h3nr_r[f % 2][:], h3n[:])

            # --- VQ stages (element e) ---
            def st_vq_G(e):
                for cb in ("s", "n"):
                    v = vq[cb]
                    mdim = v["mdim"]
                    h3cbr = (h3sr_r if cb == "s" else h3nr_r)[e % 2]
                    psG = nxt("psC", psC_r)
                    nc.tensor.matmul(psG[0:mdim, :], v["wgr"][:], h3cbr[:])
                    ecb = (e_s_r if cb == "s" else e_n_r)[e % 2]
                    nc.scalar.activation(ecb[:], psG[0:mdim, :], AF.Exp,
                                         scale=SCALE)

            def st_vq_GO(e):
                go = go_r[0]
                for cb in ("s", "n"):
                    v = vq[cb]
                    mdim = v["mdim"]
                    h3cb = (h3s_r if cb == "s" else h3n_r)[e % 2]
                    goff = 0 if cb == "s" else 4 * NUM_M
                    for j in range(4):
                        nc.tensor.matmul(
                            psGO_t[:, goff + j * mdim:goff + (j + 1) * mdim],
                            h3cb[:, 128 * j:128 * (j + 1)], v["wg"][:])
                    nc.scalar.copy(go[:, goff:goff + 4 * mdim],
                                   psGO_t[:, goff:goff + 4 * mdim])

            def st_vq_U(e):
                for cb in ("s", "n"):
                    v = vq[cb]
                    ecb = (e_s_r if cb == "s" else e_n_r)[e % 2]
                    psU = nxt("psC", psC_r)
                    nc.tensor.matmul(psU[0:33, :], v["mTar"][:], ecb[:])
                    ci_ = 0 if cb == "s" else 32
                    rcp = rcpf_r[e % 2][ci_:ci_ + 1, :]
                    rcpr = rcprr_r[e % 2][ci_:ci_ + 1, :]
                    nc.vector.reciprocal(rcp, psU[32:33, :])
                    nc.vector.tensor_copy(rcpr, rcp)
                    v["psU_live"] = psU

            def st_vq_bc(e):
                for cb in ("s", "n"):
                    v = vq[cb]
                    ci_ = 0 if cb == "s" else 32
                    rcpr = rcprr_r[e % 2][ci_:ci_ + 1, :]
                    psR = nxt("psC", psC_r)
                    nc.tensor.matmul(psR[0:32, :],
                                     ones32r[ci_:ci_ + 1, :], rcpr)
                    v["psR_live"] = psR

            def st_vq_tail(e):
                em = e % 16
                ztq = nxt("ztmp", ztmp_r)
                go = go_r[0]
                for ci, cb in enumerate(("s", "n")):
                    v = vq[cb]
                    mdim = v["mdim"]
                    qun = qun_r[0]
                    nc.scalar.copy(qun[:], v["psU_live"][0:32, :])
                    nc.vector.tensor_mul(ztq[32 * ci:32 * ci + 32, 4:T + 4],
                                         qun[:], v["psR_live"][0:32, :])
                    goff = 0 if cb == "s" else 4 * NUM_M
                    mx = mx_r[e % 2]
                    for j in range(4):
                        nc.vector.max(mx[:], go[:, goff + j * mdim:
                                                goff + (j + 1) * mdim])
                        nc.vector.max_index(
                            idx_r[cb][(e // 16) % 2][:, em, j, :], mx[:],
                            go[:, goff + j * mdim:goff + (j + 1) * mdim])
                zq = zq_r[e % 2]
                nc.vector.tensor_copy(zq[0:D, 0:T + 8], ztq[:, 0:T + 8])
                nc.gpsimd.tensor_copy(zq[D:128, 0:T + 8], ztq[:, 1:T + 9])
                if em == 15:
                    e0 = e - 15
                    for cb, dram in (("s", idx_s_d), ("n", idx_n_d)):
                        nc.sync.dma_start(
                            dram[e0:e0 + 16, :].rearrange(
                                "e (j p) -> p e j", p=128),
                            idx_r[cb][(e // 16) % 2][:, :, :, 0].bitcast(I32))

            # --- decoder stages (element e) ---
            def _zdec(e, i):
                return (zq_r[e % 2], zq_r[(e + 1) % 2], zq_r[e % 2])[i]

            def st_dec_conv1(e, i):
                z_dec = _zdec(e, i)
                ps = nxt("psC", psC_r)
                for p in range(5):
                    nc.tensor.matmul(ps[:], wdecr[(i, 0, p)][:],
                                     z_dec[:, 2 * p:2 * p + T],
                                     start=(p == 0), stop=(p == 4))
                ztd = nxt("ztmp", ztmp_r)
                nc.scalar.activation(ztd[:, 4:T + 4], ps[:], AF.Relu,
                                     bias=bdec[(i, 0)][:])
                zdm = zdm_r[e % 2]
                nc.vector.tensor_copy(zdm[0:D, 0:T + 8], ztd[:, 0:T + 8])
                nc.gpsimd.tensor_copy(zdm[D:128, 0:T + 8], ztd[:, 1:T + 9])

            def st_dec_conv2(e, i):
                z_dec = _zdec(e, i)
                zdm = zdm_r[e % 2]
                ps = nxt("psC", psC_r)
                nc.tensor.matmul(ps[:], id64r[:], z_dec[0:D, 4:T + 4],
                                 start=True, stop=False)
                for p in range(5):
                    nc.tensor.matmul(ps[:], wdecr[(i, 1, p)][:],
                                     zdm[:, 2 * p:2 * p + T],
                                     start=False, stop=(p == 4),
                                     skip_group_check=True)
                if i < 2:
                    ztd2 = nxt("ztmp", ztmp_r)
                    nc.scalar.activation(ztd2[:, 4:T + 4], ps[:], AF.Relu,
                                         bias=bdec[(i, 1)][:])
                    zn = _zdec(e, i + 1)
                    nc.vector.tensor_copy(zn[0:D, 0:T + 8], ztd2[:, 0:T + 8])
                    nc.gpsimd.tensor_copy(zn[D:128, 0:T + 8], ztd2[:, 1:T + 9])
                else:
                    ztl = ztail_r[e % 2]
                    nc.scalar.activation(ztl[:], ps[:], AF.Relu,
                                         bias=bdec[(i, 1)][:])

            def st_tail(e):
                ztl = ztail_r[e % 2]
                for tb in range(4):
                    nc.tensor.transpose(psT_t[:, tb * D:(tb + 1) * D],
                                        ztl[:, 128 * tb:128 * (tb + 1)],
                                        id64f[:])
                for tb in range(4):
                    nc.vector.tensor_copy(
                        ZT[tb][:, :, e, :],
                        psT_t[:, tb * D:(tb + 1) * D].rearrange(
                            "p (st c) -> p st c", st=2))

            # ------------- software-pipelined element loop ---------------
            def dma_x(f):
                nc.sync.dma_start(x9_r[f % 3][:], xcol_d[f])

            dma_x(0)
            if N_ELEM > 1:
                dma_x(1)
            st_enc0_mm1(0)
            st_enc0_conv2(0)
            for i in range(3):
                st_block_conv1(0, i)
                st_block_conv2(0, i)

            for e in range(N_ELEM):
                nxt_e = e + 1 if e + 1 < N_ELEM else None
                if nxt_e is not None and nxt_e + 1 < N_ELEM:
                    dma_x(nxt_e + 1)
                st_vq_G(e)
                if nxt_e is not None:
                    st_enc0_mm1(nxt_e)
                st_vq_GO(e)
                st_vq_U(e)
                if nxt_e is not None:
                    st_enc0_conv2(nxt_e)
                st_vq_bc(e)
                st_vq_tail(e)
                for i in range(3):
                    if nxt_e is not None:
                        st_block_conv1(nxt_e, i)
                    st_dec_conv1(e, i)
                    if nxt_e is not None:
                        st_block_conv2(nxt_e, i)
                    st_dec_conv2(e, i)
                st_tail(e)

            # flush partial idx ring (only when N_ELEM % 16 != 0)
            rem = N_ELEM % 16
            if rem:
                e0 = N_ELEM - rem
                for cb, dram in (("s", idx_s_d), ("n", idx_n_d)):
                    nc.sync.dma_start(
                        dram[e0:e0 + rem, :].rearrange(
                            "e (j p) -> p e j", p=128),
                        idx_r[cb][(e0 // 16) % 2][:, 0:rem, :, 0].bitcast(I32))

            # ------------- FC phase (bf16) -------------------------------
            fcw_f = [zpool.tile([128, 512], F32, tag=f"fcwf{i}",
                                name=f"fcwf{i}") for i in range(4)]
            fcw_b = [zpool.tile([128, 512], BF16, tag=f"fcwb{i}",
                                name=f"fcwb{i}") for i in range(4)]
            sf_r = [zpool.tile([128, 256], F32, tag=f"sf{i}", name=f"sf{i}")
                    for i in range(2)]
            sft_r = [zpool.tile([128, 128], F32, tag=f"sft{i}", name=f"sft{i}")
                     for i in range(4)]
            psF4 = psW_r + [pp.tile([128, 256], F32, tag=f"psC{i + 2}",
                                    name=f"psW{i + 2}") for i in range(2)]
            for c in range(128):
                wf = fcw_f[c % 4]
                wb = fcw_b[c % 4]
                nc.sync.dma_start(wf[:], fcwT_d[c * 128:(c + 1) * 128, :])
                nc.vector.tensor_copy(wb[:], wf[:])
                for ob in range(4):
                    nc.tensor.matmul(
                        psF4[ob][:], wb[:, ob * 128:(ob + 1) * 128],
                        ZT[c % 4][:, :, :, c // 4],
                        start=(c == 0), stop=(c == 127))
            for ob in range(4):
                sf = sf_r[ob % 2]
                nc.scalar.activation(sf[:], psF4[ob][:], AF.Tanh,
                                     bias=fcb[:, ob:ob + 1])
                for st, dram in ((0, s_hat_d), (1, n_hat_d)):
                    psT2 = psT_t if st == 0 else psGO_t
                    nc.tensor.transpose(psT2[:, 0:128],
                                        sf[:, st * 128:(st + 1) * 128],
                                        id128f[:])
                    sft = sft_r[ob % 2 * 2 + st]
                    nc.scalar.copy(sft[:], psT2[:, 0:128])
                    nc.sync.dma_start(
                        dram[0:EPC, ob * 128:(ob + 1) * 128], sft[:])

    nc.compile()
    return nc


def _prep_host(inputs):
    """Host-side layout transforms (pad / im2col / transpose / stack only)."""
    f = np.float32
    x = np.asarray(inputs["x"], f)
    xpad = np.pad(x, ((0, 0), (4, 4)))
    xcol = np.ascontiguousarray(
        np.lib.stride_tricks.sliding_window_view(xpad, T, axis=1))
    assert xcol.shape == (B, KW, T), xcol.shape  # xcol[b,k,t] = xpad[b,k+t]

    w1 = np.asarray(inputs["enc0_w1"], f)     # (64,1,9)
    ws = np.asarray(inputs["enc0_ws"], f)     # (64,1,1)
    w0c1 = np.zeros((KW, 128), f)
    w0c1[:, 0:D] = w1[:, 0, :].T
    w0c1[4, D:128] = ws[:, 0, 0]

    def pairs(w):  # (Cout,Cin,9) -> (5, 2*Cin, Cout)
        co, ci, _ = w.shape
        out = np.zeros((5, 2 * ci, co), f)
        for p in range(5):
            for j in range(2):
                kk = 2 * p + j
                if kk < KW:
                    out[p, j * ci:(j + 1) * ci, :] = w[:, :, kk].T
        return out

    w0c2 = pairs(np.asarray(inputs["enc0_w2"], f))
    wenc = np.stack([np.stack([pairs(np.asarray(inputs["enc_w"][i, l], f))
                               for l in range(2)]) for i in range(3)])

    def dec_pairs(w):  # (32,32,9) -> (5, 128, 64) stream-stacked block-diag
        out = np.zeros((5, 128, D), f)
        pw = pairs(w)  # (5, 64, 32)
        for p in range(5):
            for j in range(2):
                blk = pw[p, j * 32:(j + 1) * 32, :]  # (ci, co)
                for st in range(2):
                    out[p, j * 64 + st * 32:j * 64 + (st + 1) * 32,
                        st * 32:(st + 1) * 32] = blk
        return out

    wdec = np.stack([np.stack([dec_pairs(np.asarray(inputs["dec_w"][i, l], f))
                               for l in range(2)]) for i in range(3)])
    bdec = np.stack([np.stack([np.tile(np.asarray(inputs["dec_b"][i, l], f), 2)
                               for l in range(2)]) for i in range(3)])

    fcwT = np.ascontiguousarray(np.asarray(inputs["fc1_w"], f).T)
    means_s = np.asarray(inputs["means_s"], f)
    means_n = np.asarray(inputs["means_n"], f)

    common = dict(
        w0c1=w0c1, b1e0=np.asarray(inputs["enc0_b1"], f)[:, None], w0c2=w0c2,
        b2e0=np.asarray(inputs["enc0_b2"], f)[:, None],
        bse0=np.asarray(inputs["enc0_bs"], f)[:, None],
        wenc=wenc, benc=np.asarray(inputs["enc_b"], f)[..., None],
        wdec=wdec, bdec=bdec[..., None], fcwT=fcwT,
        fcb=np.asarray(inputs["fc1_b"], f),
        means_s=means_s, meansT_s=np.ascontiguousarray(means_s.T),
        means_n=means_n, meansT_n=np.ascontiguousarray(means_n.T),
    )
    in_maps = []
    for c in range(NCORES):
        m = dict(common)
        m["xcol"] = np.ascontiguousarray(xcol[c * EPC:(c + 1) * EPC])
        in_maps.append(m)
    return in_maps


TRACE = False
TRACE_DIR = None
LAST_RESULTS = None


def kernel(**inputs):
    global _COMPILED, LAST_RESULTS
    from concourse.bass_utils import run_bass_kernel_spmd

    if _COMPILED is None:
        _COMPILED = _build()
    in_maps = _prep_host(inputs)
    kw = {}
    if TRACE:
        kw = dict(trace=True, tmpdir=TRACE_DIR)
    res = run_bass_kernel_spmd(_COMPILED, in_maps, core_ids=list(range(NCORES)),
                               **kw)
    LAST_RESULTS = res
    s_hat = np.concatenate([res.results[c]["s_hat"] for c in range(NCORES)], 0)
    n_hat = np.concatenate([res.results[c]["n_hat"] for c in range(NCORES)], 0)
    idx_s = np.concatenate([res.results[c]["idx_s"] for c in range(NCORES)], 0)
    idx_n = np.concatenate([res.results[c]["idx_n"] for c in range(NCORES)], 0)
    return s_hat, n_hat, idx_s.astype(np.int32), idx_n.astype(np.int32)


# revision 25
# speedup vs baseline: 1.2488x; 1.2488x over previous
"""Trainium2 Bass kernel for the vq_codebook autoencoder (nn_AE_control_54546084659681).

Data-parallel across 8 NeuronCores: each core processes 128 of the 1024 batch
elements; all weights are replicated. No collectives (forward only).

Encoder convs run as split-bf16 matmuls (hi/lo decomposition, 3 products per
tap-pair, f32 PSUM accumulate - ~1e-5 relative accuracy, protecting the VQ
argmin indices, at 1 cycle/row). The VQ argmax scores are computed in plain
f32 (exact). Softmax/q and the decoder run in f32r (1 cycle/row), and the
final 512x16384 FC runs in bf16. The element loop is software-pipelined:
element e's VQ + decoder stages are emitted between element e+1's encoder
stages so the in-order PE stream always has ready matmuls.

Self-contained: hardcodes all shapes; only needs /opt/trn_rl_repo on the path.
"""
import os
import sys

sys.path.insert(0, "/opt/trn_rl_repo")

import numpy as np

T = 512
D = 64
KW = 9          # conv kernel width
B = 1024
NCORES = 8
EPC = B // NCORES          # elements per core = 128
NUM_M = 64
NUM_N = 32
SCALE = 1.0
N_ELEM = int(os.environ.get("BASS_N_ELEM", str(EPC)))  # loop count (debug aid)

_COMPILED = None


def _build():
    from concourse import bacc, tile, mybir

    F32 = mybir.dt.float32
    F32R = mybir.dt.float32r
    BF16 = mybir.dt.bfloat16
    U32 = mybir.dt.uint32
    I32 = mybir.dt.int32
    AF = mybir.ActivationFunctionType
    SUB = mybir.AluOpType.subtract

    nc = bacc.Bacc("TRN2", target_bir_lowering=False, debug=False,
                   num_devices=NCORES)

    # ---------------- DRAM parameters -----------------------------------
    def din(name, shape):
        return nc.dram_tensor(name, list(shape), F32, kind="ExternalInput").ap()

    xcol_d = din("xcol", (EPC, KW, T))            # im2col'd padded input
    w0c1_d = din("w0c1", (KW, 128))               # [conv1 | shortcut] lhsT
    b1e0_d = din("b1e0", (D, 1))
    w0c2_d = din("w0c2", (5, 128, D))             # enc0 conv2 tap-pairs
    b2e0_d = din("b2e0", (D, 1))
    bse0_d = din("bse0", (D, 1))
    wenc_d = din("wenc", (3, 2, 5, 128, D))       # enc blocks tap-pairs
    benc_d = din("benc", (3, 2, D, 1))
    wdec_d = din("wdec", (3, 2, 5, 128, D))       # dec blocks, stream-stacked
    bdec_d = din("bdec", (3, 2, D, 1))            # stacked (s|n) biases
    fcwT_d = din("fcwT", (T * D // 2, 512))       # fc1_w transposed
    fcb_d = din("fcb", (512,))
    means_s_d = din("means_s", (D // 2, NUM_M))
    meansT_s_d = din("meansT_s", (NUM_M, D // 2))
    means_n_d = din("means_n", (D // 2, NUM_N))
    meansT_n_d = din("meansT_n", (NUM_N, D // 2))

    s_hat_d = nc.dram_tensor("s_hat", [EPC, 512], F32, kind="ExternalOutput").ap()
    n_hat_d = nc.dram_tensor("n_hat", [EPC, 512], F32, kind="ExternalOutput").ap()
    idx_s_d = nc.dram_tensor("idx_s", [EPC, T], I32, kind="ExternalOutput").ap()
    idx_n_d = nc.dram_tensor("idx_n", [EPC, T], I32, kind="ExternalOutput").ap()

    id64_d = nc.inline_tensor(np.eye(D, dtype=np.float32), name="id64")
    id128_d = nc.inline_tensor(np.eye(128, dtype=np.float32), name="id128")

    with tile.TileContext(nc) as tc:
        with (
            tc.tile_pool(name="wpool", bufs=1) as wpool,
            tc.tile_pool(name="zpool", bufs=1) as zpool,
            tc.tile_pool(name="iopool", bufs=1) as iopool,
            tc.tile_pool(name="psum", bufs=1, space="PSUM") as pp,
        ):
            def wtile(shape, dtype, tag):
                return wpool.tile(list(shape), dtype, tag=tag, name=tag)

            # ------------- weights: DMA f32, build bf16 hi/lo ------------
            wstage = [wtile((128, 128), F32, f"wstage{i}") for i in range(3)]
            _ws = [0]

            def wst():
                t_ = wstage[_ws[0] % 3]
                _ws[0] += 1
                return t_

            def split_pair(shape, tag, src_ap):
                tf = wst()
                nc.sync.dma_start(tf[0:shape[0], 0:shape[1]], src_ap)
                hi = wtile(shape, BF16, tag + "h")
                lo = wtile(shape, BF16, tag + "l")
                nc.vector.tensor_copy(hi[:], tf[0:shape[0], 0:shape[1]])
                nc.vector.tensor_tensor(lo[:], tf[0:shape[0], 0:shape[1]],
                                        hi[:], op=SUB)
                return hi, lo

            w0c1h, w0c1l = split_pair((KW, 128), "w0c1", w0c1_d[:])

            w0c2h, w0c2l = [], []
            for p in range(5):
                h_, l_ = split_pair((128, D), f"w0c2{p}", w0c2_d[p])
                w0c2h.append(h_)
                w0c2l.append(l_)

            wench, wencl = {}, {}
            for i in range(3):
                for l in range(2):
                    for p in range(5):
                        h_, l_ = split_pair((128, D), f"we{i}{l}{p}",
                                            wenc_d[i, l, p])
                        wench[(i, l, p)] = h_
                        wencl[(i, l, p)] = l_

            wdecr = {}
            for i in range(3):
                for l in range(2):
                    for p in range(5):
                        tf = wst()
                        nc.sync.dma_start(tf[:, 0:D], wdec_d[i, l, p])
                        tr = wtile((128, D), F32R, f"wdr{i}{l}{p}")
                        nc.vector.tensor_copy(tr[:], tf[:, 0:D])
                        wdecr[(i, l, p)] = tr

            id64f = wtile((D, D), F32, "id64f")
            nc.sync.dma_start(id64f[:], id64_d.ap()[:])
            id64r = wtile((D, D), F32R, "id64r")
            nc.vector.tensor_copy(id64r[:], id64f[:])
            id64b = wtile((D, D), BF16, "id64b")
            nc.vector.tensor_copy(id64b[:], id64f[:])
            id128f = wtile((128, 128), F32, "id128f")
            nc.sync.dma_start(id128f[:], id128_d.ap()[:])

            def bias_tile(src_ap, n, tag):
                t_ = wtile((n, 1), F32, tag)
                nc.sync.dma_start(t_[:], src_ap)
                return t_

            b1e0 = bias_tile(b1e0_d, D, "b1e0")
            b2e0 = bias_tile(b2e0_d, D, "b2e0")
            bse0 = bias_tile(bse0_d, D, "bse0")
            b2bs = wtile((D, 1), F32, "b2bs")
            nc.vector.tensor_add(b2bs[:], b2e0[:], bse0[:])
            benc = {}
            bdec = {}
            for i in range(3):
                for l in range(2):
                    benc[(i, l)] = bias_tile(benc_d[i, l], D, f"benc{i}{l}")
                    bdec[(i, l)] = bias_tile(bdec_d[i, l], D, f"bdec{i}{l}")
            fcb = wtile((128, 4), F32, "fcb")
            nc.sync.dma_start(fcb[:], fcb_d.rearrange("(ob p) -> p ob", p=128))

            # ------------- VQ codebook prep ------------------------------
            vq = {}
            for cb, (mdim, mns_d, mnsT_d) in (
                ("s", (NUM_M, means_s_d, meansT_s_d)),
                ("n", (NUM_N, means_n_d, meansT_n_d)),
            ):
                mt = wtile((mdim, 32), F32, f"mt_{cb}")
                nc.sync.dma_start(mt[:], mnsT_d[:])
                mns = wtile((32, mdim), F32, f"mns_{cb}")
                nc.sync.dma_start(mns[:], mns_d[:])
                wg = wtile((33, mdim), F32, f"wg_{cb}")
                nc.vector.tensor_scalar_mul(wg[0:32, :], mns[:], 2.0)
                sq = wtile((mdim, 32), F32, f"sq_{cb}")
                nc.vector.tensor_mul(sq[:], mt[:], mt[:])
                m2 = wtile((mdim, 1), F32, f"m2_{cb}")
                nc.vector.reduce_sum(m2[:], sq[:], axis=mybir.AxisListType.X)
                nm2 = wtile((mdim, 1), F32, f"nm2_{cb}")
                nc.vector.tensor_scalar_mul(nm2[:], m2[:], -1.0)
                pt_ = pp.tile([1, mdim], F32, tag="psA0", name="prep_t")
                nc.tensor.transpose(pt_[:], nm2[:], id64f[0:mdim, 0:mdim])
                nc.scalar.copy(wg[32:33, :], pt_[:])
                wgr = wtile((33, mdim), F32R, f"wgr_{cb}")
                nc.vector.tensor_copy(wgr[:], wg[:])
                mTa = wtile((mdim, 33), F32, f"mTa_{cb}")
                nc.vector.tensor_copy(mTa[:, 0:32], mt[:])
                nc.vector.memset(mTa[:, 32:33], 1.0)
                mTar = wtile((mdim, 33), F32R, f"mTar_{cb}")
                nc.vector.tensor_copy(mTar[:], mTa[:])
                vq[cb] = dict(mdim=mdim, wg=wg, wgr=wgr, mTar=mTar)

            ones32f = wtile((33, 32), F32, "ones32f")
            nc.vector.memset(ones32f[0:1, :], 1.0)
            nc.vector.memset(ones32f[32:33, :], 1.0)
            ones32r = wtile((33, 32), F32R, "ones32r")
            nc.vector.tensor_copy(ones32r[0:1, :], ones32f[0:1, :])
            nc.vector.tensor_copy(ones32r[32:33, :], ones32f[32:33, :])

            # ------------- ring tiles ------------------------------------
            def ring(n, shape, dtype, tag):
                return [zpool.tile(list(shape), dtype, tag=f"{tag}{i}",
                                   name=f"{tag}{i}") for i in range(n)]

            ZW = T + 9            # 521
            x9_r = ring(3, (KW, T), F32, "x9")
            x9h_r = ring(2, (KW, T), BF16, "x9h")
            x9l_r = ring(2, (KW, T), BF16, "x9l")

            def zpair_ring(tag):
                return (ring(2, (128, ZW), BF16, tag + "h"),
                        ring(2, (128, ZW), BF16, tag + "l"))

            zA_r = zpair_ring("zA")     # enc0 conv2 input
            zB_r = zpair_ring("zB")     # enc0 out / block0 in
            z1_r = zpair_ring("z1")     # block0 out
            z2_r = zpair_ring("z2")     # block1 out
            zM_r = zpair_ring("zM")     # block mids
            ztf_r = ring(3, (D, ZW), F32, "ztf")      # enc evict staging
            h3s_r = ring(2, (33, T), F32, "h3s")
            h3n_r = ring(2, (33, T), F32, "h3n")
            h3sr_r = ring(2, (33, T), F32R, "h3sr")
            h3nr_r = ring(2, (33, T), F32R, "h3nr")
            e_s_r = ring(2, (NUM_M, T), F32R, "es")
            e_n_r = ring(2, (NUM_N, T), F32R, "en")
            qun_r = ring(1, (32, T), F32, "qun")
            rcpf_r = ring(2, (33, T), F32, "rcpf")
            rcprr_r = ring(2, (33, T), F32R, "rcprr")
            go_r = ring(1, (128, 4 * NUM_M + 4 * NUM_N), F32, "go")
            mx_r = ring(2, (128, 8), F32, "mx")
            ztmp_r = ring(3, (D, ZW), F32, "ztmp")    # q/dec evict staging
            zq_r = ring(2, (128, T + 8), F32R, "zq")  # dec conv inputs
            zdm_r = ring(2, (128, T + 8), F32R, "zdm")
            ztail_r = ring(2, (D, T), F32, "ztail")
            idx_r = {"s": ring(2, (128, 16, 4, 8), U32, "idxs"),
                     "n": ring(2, (128, 16, 4, 8), U32, "idxn")}

            ZT = [iopool.tile([128, 2, EPC, 32], BF16, tag=f"zt{tb}",
                              name=f"zt{tb}") for tb in range(4)]

            for z in ztf_r + ztmp_r:
                nc.gpsimd.memset(z[:, 0:4], 0.0)
                nc.gpsimd.memset(z[:, T + 4:ZW], 0.0)
            for h in h3s_r + h3n_r:
                nc.gpsimd.memset(h[32:33, :], 1.0)

            psA_r = [pp.tile([128, T], F32, tag=f"psA{i}", name=f"psA{i}")
                     for i in range(2)]
            psC_r = [pp.tile([D, T], F32, tag=f"psC{i}", name=f"psC{i}")
                     for i in range(4)]
            psGO_t = pp.tile([128, 4 * NUM_M + 4 * NUM_N], F32, tag="psGO",
                             name="psGO")
            psT_t = pp.tile([128, 256], F32, tag="psT", name="psT")
            psW_r = [pp.tile([128, 256], F32, tag=f"psC{i}", name=f"psW{i}")
                     for i in range(2)]
            _cnt = {}

            def nxt(name, lst):
                i = _cnt.get(name, 0)
                _cnt[name] = i + 1
                return lst[i % len(lst)]

            # ------------- stage helpers ---------------------------------
            def build_split(ps_in, bias, pair, f):
                """relu+bias evict -> f32 staging -> bf16 hi/lo doubled-shift."""
                ztf = nxt("ztf", ztf_r)
                nc.scalar.activation(ztf[:, 4:T + 4], ps_in, AF.Relu,
                                     bias=bias[:])
                zh, zl = pair[0][f % 2], pair[1][f % 2]
                nc.vector.tensor_copy(zh[0:D, 0:ZW], ztf[:, 0:ZW])
                nc.vector.tensor_copy(zh[D:128, 0:260], ztf[:, 1:261])
                nc.gpsimd.tensor_copy(zh[D:128, 260:ZW - 1], ztf[:, 261:ZW])
                nc.vector.tensor_tensor(zl[0:D, 0:ZW], ztf[:, 0:ZW],
                                        zh[0:D, 0:ZW], op=SUB)
                nc.gpsimd.tensor_copy(zl[D:128, 0:ZW - 1], zl[0:D, 1:ZW])

            def conv15(wh, wl, zh, zl, ps, first_start, last_stop):
                prods = ([(wh[p], zh, p) for p in range(5)]
                         + [(wl[p], zh, p) for p in range(5)]
                         + [(wh[p], zl, p) for p in range(5)])
                for k, (lhs, rhs, p) in enumerate(prods):
                    nc.tensor.matmul(
                        ps, lhs[:], rhs[:, 2 * p:2 * p + T],
                        start=(k == 0 and first_start),
                        stop=(k == 14 and last_stop),
                        skip_group_check=True)

            # --- encoder stages (element f = e+1 pipelined) ---
            def st_enc0_mm1(f):
                x9 = x9_r[f % 3]
                x9h, x9l = x9h_r[f % 2], x9l_r[f % 2]
                nc.vector.tensor_copy(x9h[:], x9[:])
                nc.vector.tensor_tensor(x9l[:], x9[:], x9h[:], op=SUB)
                psA = psA_r[f % 2]
                nc.tensor.matmul(psA[:], w0c1h[:], x9h[:], start=True,
                                 stop=False)
                nc.tensor.matmul(psA[:], w0c1l[:], x9h[:], start=False,
                                 stop=False, skip_group_check=True)
                nc.tensor.matmul(psA[:], w0c1h[:], x9l[:], start=False,
                                 stop=False, skip_group_check=True)
                build_split(psA[0:D, :], b1e0, zA_r, f)

            def st_enc0_conv2(f):
                psA = psA_r[f % 2]
                zh, zl = zA_r[0][f % 2], zA_r[1][f % 2]
                conv15(w0c2h, w0c2l, zh, zl, psA[D:128, :], False, True)
                build_split(psA[D:128, :], b2bs, zB_r, f)

            def _zin(f, i):
                src = (zB_r, z1_r, z2_r)[i]
                return src[0][f % 2], src[1][f % 2]

            def st_block_conv1(f, i):
                zh, zl = _zin(f, i)
                ps = nxt("psC", psC_r)
                conv15([wench[(i, 0, p)] for p in range(5)],
                       [wencl[(i, 0, p)] for p in range(5)],
                       zh, zl, ps[:], True, True)
                build_split(ps[:], benc[(i, 0)], zM_r, f)

            def st_block_conv2(f, i):
                zmh, zml = zM_r[0][f % 2], zM_r[1][f % 2]
                zih, zil = _zin(f, i)
                ps = nxt("psC", psC_r)
                conv15([wench[(i, 1, p)] for p in range(5)],
                       [wencl[(i, 1, p)] for p in range(5)],
                       zmh, zml, ps[:], True, False)
                nc.tensor.matmul(ps[:], id64b[:], zih[0:D, 4:T + 4],
                                 start=False, stop=False,
                                 skip_group_check=True)
                nc.tensor.matmul(ps[:], id64b[:], zil[0:D, 4:T + 4],
                                 start=False, stop=True,
                                 skip_group_check=True)
                if i < 2:
                    build_split(ps[:], benc[(i, 1)], (z1_r, z2_r)[i], f)
                else:
                    h3s, h3n = h3s_r[f % 2], h3n_r[f % 2]
                    nc.scalar.activation(h3s[0:32, :], ps[0:32, :], AF.Relu,
                                         bias=benc[(i, 1)][0:32, :])
                    nc.scalar.activation(h3n[0:32, :], ps[32:D, :], AF.Relu,
                                         bias=benc[(i, 1)][32:D, :])
                    nc.vector.tensor_copy(h3sr_r[f % 2][:], h3s[:])
                    nc.vector.tensor_copy(h3nr_r[f % 2][:], h3n[:])

            # --- VQ stages (element e) ---
            def st_vq_G(e):
                for cb in ("s", "n"):
                    v = vq[cb]
                    mdim = v["mdim"]
                    h3cbr = (h3sr_r if cb == "s" else h3nr_r)[e % 2]
                    psG = nxt("psC", psC_r)
                    nc.tensor.matmul(psG[0:mdim, :], v["wgr"][:], h3cbr[:])
                    ecb = (e_s_r if cb == "s" else e_n_r)[e % 2]
                    nc.scalar.activation(ecb[:], psG[0:mdim, :], AF.Exp,
                                         scale=SCALE)

            def st_vq_GO(e):
                go = go_r[0]
                for cb in ("s", "n"):
                    v = vq[cb]
                    mdim = v["mdim"]
                    h3cb = (h3s_r if cb == "s" else h3n_r)[e % 2]
                    goff = 0 if cb == "s" else 4 * NUM_M
                    for j in range(4):
                        nc.tensor.matmul(
                            psGO_t[:, goff + j * mdim:goff + (j + 1) * mdim],
                            h3cb[:, 128 * j:128 * (j + 1)], v["wg"][:])
                    nc.scalar.copy(go[:, goff:goff + 4 * mdim],
                                   psGO_t[:, goff:goff + 4 * mdim])

            def st_vq_U(e):
                for cb in ("s", "n"):
                    v = vq[cb]
                    ecb = (e_s_r if cb == "s" else e_n_r)[e % 2]
                    psU = nxt("psC", psC_r)
                    nc.tensor.matmul(psU[0:33, :], v["mTar"][:], ecb[:])
                    ci_ = 0 if cb == "s" else 32
                    rcp = rcpf_r[e % 2][ci_:ci_ + 1, :]
                    rcpr = rcprr_r[e % 2][ci_:ci_ + 1, :]
                    nc.vector.reciprocal(rcp, psU[32:33, :])
                    nc.vector.tensor_copy(rcpr, rcp)
                    v["psU_live"] = psU

            def st_vq_bc(e):
                for cb in ("s", "n"):
                    v = vq[cb]
                    ci_ = 0 if cb == "s" else 32
                    rcpr = rcprr_r[e % 2][ci_:ci_ + 1, :]
                    psR = nxt("psC", psC_r)
                    nc.tensor.matmul(psR[0:32, :],
                                     ones32r[ci_:ci_ + 1, :], rcpr)
                    v["psR_live"] = psR

            def st_vq_tail(e):
                em = e % 16
                ztq = nxt("ztmp", ztmp_r)
                go = go_r[0]
                for ci, cb in enumerate(("s", "n")):
                    v = vq[cb]
                    mdim = v["mdim"]
                    qun = qun_r[0]
                    nc.scalar.copy(qun[:], v["psU_live"][0:32, :])
                    nc.vector.tensor_mul(ztq[32 * ci:32 * ci + 32, 4:T + 4],
                                         qun[:], v["psR_live"][0:32, :])
                    goff = 0 if cb == "s" else 4 * NUM_M
                    mx = mx_r[e % 2]
                    for j in range(4):
                        nc.vector.max(mx[:], go[:, goff + j * mdim:
                                                goff + (j + 1) * mdim])
                        nc.vector.max_index(
                            idx_r[cb][(e // 16) % 2][:, em, j, :], mx[:],
                            go[:, goff + j * mdim:goff + (j + 1) * mdim])
                zq = zq_r[e % 2]
                nc.vector.tensor_copy(zq[0:D, 0:T + 8], ztq[:, 0:T + 8])
                nc.gpsimd.tensor_copy(zq[D:128, 0:T + 8], ztq[:, 1:T + 9])
                if em == 15:
                    e0 = e - 15
                    for cb, dram in (("s", idx_s_d), ("n", idx_n_d)):
                        nc.sync.dma_start(
                            dram[e0:e0 + 16, :].rearrange(
                                "e (j p) -> p e j", p=128),
                            idx_r[cb][(e // 16) % 2][:, :, :, 0].bitcast(I32))

            # --- decoder stages (element e) ---
            def _zdec(e, i):
                return (zq_r[e % 2], zq_r[(e + 1) % 2], zq_r[e % 2])[i]

            def st_dec_conv1(e, i):
                z_dec = _zdec(e, i)
                ps = nxt("psC", psC_r)
                for p in range(5):
                    nc.tensor.matmul(ps[:], wdecr[(i, 0, p)][:],
                                     z_dec[:, 2 * p:2 * p + T],
                                     start=(p == 0), stop=(p == 4))
                ztd = nxt("ztmp", ztmp_r)
                nc.scalar.activation(ztd[:, 4:T + 4], ps[:], AF.Relu,
                                     bias=bdec[(i, 0)][:])
                zdm = zdm_r[e % 2]
                nc.vector.tensor_copy(zdm[0:D, 0:T + 8], ztd[:, 0:T + 8])
                nc.gpsimd.tensor_copy(zdm[D:128, 0:T + 8], ztd[:, 1:T + 9])

            def st_dec_conv2(e, i):
                z_dec = _zdec(e, i)
                zdm = zdm_r[e % 2]
                ps = nxt("psC", psC_r)
                nc.tensor.matmul(ps[:], id64r[:], z_dec[0:D, 4:T + 4],
                                 start=True, stop=False)
                for p in range(5):
                    nc.tensor.matmul(ps[:], wdecr[(i, 1, p)][:],
                                     zdm[:, 2 * p:2 * p + T],
                                     start=False, stop=(p == 4),
                                     skip_group_check=True)
                if i < 2:
                    ztd2 = nxt("ztmp", ztmp_r)
                    nc.scalar.activation(ztd2[:, 4:T + 4], ps[:], AF.Relu,
                                         bias=bdec[(i, 1)][:])
                    zn = _zdec(e, i + 1)
                    nc.vector.tensor_copy(zn[0:D, 0:T + 8], ztd2[:, 0:T + 8])
                    nc.gpsimd.tensor_copy(zn[D:128, 0:T + 8], ztd2[:, 1:T + 9])
                else:
                    ztl = ztail_r[e % 2]
                    nc.scalar.activation(ztl[:], ps[:], AF.Relu,
                                         bias=bdec[(i, 1)][:])

            def st_tail(e):
                ztl = ztail_r[e % 2]
                for tb in range(4):
                    nc.tensor.transpose(psT_t[:, tb * D:(tb + 1) * D],
                                        ztl[:, 128 * tb:128 * (tb + 1)],
                                        id64f[:])
                for tb in range(4):
                    nc.vector.tensor_copy(
                        ZT[tb][:, :, e, :],
                        psT_t[:, tb * D:(tb + 1) * D].rearrange(
                            "p (st c) -> p st c", st=2))

            # ------------- software-pipelined element loop ---------------
            def dma_x(f):
                nc.sync.dma_start(x9_r[f % 3][:], xcol_d[f])

            dma_x(0)
            if N_ELEM > 1:
                dma_x(1)
            st_enc0_mm1(0)
            st_enc0_conv2(0)
            for i in range(3):
                st_block_conv1(0, i)
                st_block_conv2(0, i)

            for e in range(N_ELEM):
                nxt_e = e + 1 if e + 1 < N_ELEM else None
                if nxt_e is not None and nxt_e + 1 < N_ELEM:
                    dma_x(nxt_e + 1)
                st_vq_G(e)
                if nxt_e is not None:
                    st_enc0_mm1(nxt_e)
                st_vq_GO(e)
                st_vq_U(e)
                if nxt_e is not None:
                    st_enc0_conv2(nxt_e)
                st_vq_bc(e)
                st_vq_tail(e)
                for i in range(3):
                    if nxt_e is not None:
                        st_block_conv1(nxt_e, i)
                    st_dec_conv1(e, i)
                    if nxt_e is not None:
                        st_block_conv2(nxt_e, i)
                    st_dec_conv2(e, i)
                st_tail(e)

            # flush partial idx ring (only when N_ELEM % 16 != 0)
            rem = N_ELEM % 16
            if rem:
                e0 = N_ELEM - rem
                for cb, dram in (("s", idx_s_d), ("n", idx_n_d)):
                    nc.sync.dma_start(
                        dram[e0:e0 + rem, :].rearrange(
                            "e (j p) -> p e j", p=128),
                        idx_r[cb][(e0 // 16) % 2][:, 0:rem, :, 0].bitcast(I32))

            # ------------- FC phase (bf16) -------------------------------
            fcw_f = [zpool.tile([128, 512], F32, tag=f"fcwf{i}",
                                name=f"fcwf{i}") for i in range(4)]
            fcw_b = [zpool.tile([128, 512], BF16, tag=f"fcwb{i}",
                                name=f"fcwb{i}") for i in range(4)]
            sf_r = [zpool.tile([128, 256], F32, tag=f"sf{i}", name=f"sf{i}")
                    for i in range(2)]
            sft_r = [zpool.tile([128, 128], F32, tag=f"sft{i}", name=f"sft{i}")
                     for i in range(4)]
            psF4 = psW_r + [pp.tile([128, 256], F32, tag=f"psC{i + 2}",
                                    name=f"psW{i + 2}") for i in range(2)]
            for c in range(128):
                wf = fcw_f[c % 4]
                wb = fcw_b[c % 4]
                nc.sync.dma_start(wf[:], fcwT_d[c * 128:(c + 1) * 128, :])
                nc.vector.tensor_copy(wb[:], wf[:])
                for ob in range(4):
                    nc.tensor.matmul(
                        psF4[ob][:], wb[:, ob * 128:(ob + 1) * 128],
                        ZT[c % 4][:, :, :, c // 4],
                        start=(c == 0), stop=(c == 127))
            for ob in range(4):
                sf = sf_r[ob % 2]
                nc.scalar.activation(sf[:], psF4[ob][:], AF.Tanh,
                                     bias=fcb[:, ob:ob + 1])
                for st, dram in ((0, s_hat_d), (1, n_hat_d)):
                    psT2 = psT_t if st == 0 else psGO_t
                    nc.tensor.transpose(psT2[:, 0:128],
                                        sf[:, st * 128:(st + 1) * 128],
                                        id128f[:])
                    sft = sft_r[ob % 2 * 2 + st]
                    nc.scalar.copy(sft[:], psT2[:, 0:128])
                    nc.sync.dma_start(
                        dram[0:EPC, ob * 128:(ob + 1) * 128], sft[:])

    nc.compile()
    return nc


def _prep_host(inputs):
    """Host-side layout transforms (pad / im2col / transpose / stack only)."""
    f = np.float32
    x = np.asarray(inputs["x"], f)
    xpad = np.pad(x, ((0, 0), (4, 4)))
    xcol = np.ascontiguousarray(
        np.lib.stride_tricks.sliding_window_view(xpad, T, axis=1))
    assert xcol.shape == (B, KW, T), xcol.shape  # xcol[b,k,t] = xpad[b,k+t]

    w1 = np.asarray(inputs["enc0_w1"], f)     # (64,1,9)
    ws = np.asarray(inputs["enc0_ws"], f)     # (64,1,1)
    w0c1 = np.zeros((KW, 128), f)
    w0c1[:, 0:D] = w1[:, 0, :].T
    w0c1[4, D:128] = ws[:, 0, 0]

    def pairs(w):  # (Cout,Cin,9) -> (5, 2*Cin, Cout)
        co, ci, _ = w.shape
        out = np.zeros((5, 2 * ci, co), f)
        for p in range(5):
            for j in range(2):
                kk = 2 * p + j
                if kk < KW:
                    out[p, j * ci:(j + 1) * ci, :] = w[:, :, kk].T
        return out

    w0c2 = pairs(np.asarray(inputs["enc0_w2"], f))
    wenc = np.stack([np.stack([pairs(np.asarray(inputs["enc_w"][i, l], f))
                               for l in range(2)]) for i in range(3)])

    def dec_pairs(w):  # (32,32,9) -> (5, 128, 64) stream-stacked block-diag
        out = np.zeros((5, 128, D), f)
        pw = pairs(w)  # (5, 64, 32)
        for p in range(5):
            for j in range(2):
                blk = pw[p, j * 32:(j + 1) * 32, :]  # (ci, co)
                for st in range(2):
                    out[p, j * 64 + st * 32:j * 64 + (st + 1) * 32,
                        st * 32:(st + 1) * 32] = blk
        return out

    wdec = np.stack([np.stack([dec_pairs(np.asarray(inputs["dec_w"][i, l], f))
                               for l in range(2)]) for i in range(3)])
    bdec = np.stack([np.stack([np.tile(np.asarray(inputs["dec_b"][i, l], f), 2)
                               for l in range(2)]) for i in range(3)])

    fcwT = np.ascontiguousarray(np.asarray(inputs["fc1_w"], f).T)
    means_s = np.asarray(inputs["means_s"], f)
    means_n = np.asarray(inputs["means_n"], f)

    common = dict(
        w0c1=w0c1, b1e0=np.asarray(inputs["enc0_b1"], f)[:, None], w0c2=w0c2,
        b2e0=np.asarray(inputs["enc0_b2"], f)[:, None],
        bse0=np.asarray(inputs["enc0_bs"], f)[:, None],
        wenc=wenc, benc=np.asarray(inputs["enc_b"], f)[..., None],
        wdec=wdec, bdec=bdec[..., None], fcwT=fcwT,
        fcb=np.asarray(inputs["fc1_b"], f),
        means_s=means_s, meansT_s=np.ascontiguousarray(means_s.T),
        means_n=means_n, meansT_n=np.ascontiguousarray(means_n.T),
    )
    in_maps = []
    for c in range(NCORES):
        m = dict(common)
        m["xcol"] = np.ascontiguousarray(xcol[c * EPC:(c + 1) * EPC])
        in_maps.append(m)
    return in_maps


TRACE = False
TRACE_DIR = None
LAST_RESULTS = None


def kernel(**inputs):
    global _COMPILED, LAST_RESULTS
    from concourse.bass_utils import run_bass_kernel_spmd

    if _COMPILED is None:
        _COMPILED = _build()
    in_maps = _prep_host(inputs)
    kw = {}
    if TRACE:
        kw = dict(trace=True, tmpdir=TRACE_DIR)
    res = run_bass_kernel_spmd(_COMPILED, in_maps, core_ids=list(range(NCORES)),
                               **kw)
    LAST_RESULTS = res
    s_hat = np.concatenate([res.results[c]["s_hat"] for c in range(NCORES)], 0)
    n_hat = np.concatenate([res.results[c]["n_hat"] for c in range(NCORES)], 0)
    idx_s = np.concatenate([res.results[c]["idx_s"] for c in range(NCORES)], 0)
    idx_n = np.concatenate([res.results[c]["idx_n"] for c in range(NCORES)], 0)
    return s_hat, n_hat, idx_s.astype(np.int32), idx_n.astype(np.int32)


# revision 26
# speedup vs baseline: 1.2657x; 1.0136x over previous
"""Trainium2 Bass kernel for the vq_codebook autoencoder (nn_AE_control_54546084659681).

Data-parallel across 8 NeuronCores: each core processes 128 of the 1024 batch
elements; all weights are replicated. No collectives (forward only).

Encoder convs run as split-bf16 matmuls (hi/lo decomposition, 3 products per
tap-pair, f32 PSUM accumulate - ~1e-5 relative accuracy, protecting the VQ
argmin indices, at 1 cycle/row). The VQ argmax scores are computed in plain
f32 (exact). Softmax/q and the decoder run in f32r (1 cycle/row), and the
final 512x16384 FC runs in bf16. The element loop is software-pipelined:
element e's VQ + decoder stages are emitted between element e+1's encoder
stages so the in-order PE stream always has ready matmuls.

Self-contained: hardcodes all shapes; only needs /opt/trn_rl_repo on the path.
"""
import os
import sys

sys.path.insert(0, "/opt/trn_rl_repo")

import numpy as np

T = 512
D = 64
KW = 9          # conv kernel width
B = 1024
NCORES = 8
EPC = B // NCORES          # elements per core = 128
NUM_M = 64
NUM_N = 32
SCALE = 1.0
N_ELEM = int(os.environ.get("BASS_N_ELEM", str(EPC)))  # loop count (debug aid)

_COMPILED = None


def _build():
    from concourse import bacc, tile, mybir

    F32 = mybir.dt.float32
    F32R = mybir.dt.float32r
    BF16 = mybir.dt.bfloat16
    U32 = mybir.dt.uint32
    I32 = mybir.dt.int32
    AF = mybir.ActivationFunctionType
    SUB = mybir.AluOpType.subtract

    nc = bacc.Bacc("TRN2", target_bir_lowering=False, debug=False,
                   num_devices=NCORES)

    # ---------------- DRAM parameters -----------------------------------
    def din(name, shape):
        return nc.dram_tensor(name, list(shape), F32, kind="ExternalInput").ap()

    xcol_d = din("xcol", (EPC, KW, T))            # im2col'd padded input
    w0c1_d = din("w0c1", (KW, 128))               # [conv1 | shortcut] lhsT
    b1e0_d = din("b1e0", (D, 1))
    w0c2_d = din("w0c2", (5, 128, D))             # enc0 conv2 tap-pairs
    b2e0_d = din("b2e0", (D, 1))
    bse0_d = din("bse0", (D, 1))
    wenc_d = din("wenc", (3, 2, 5, 128, D))       # enc blocks tap-pairs
    benc_d = din("benc", (3, 2, D, 1))
    wdec_d = din("wdec", (3, 2, 5, 128, D))       # dec blocks, stream-stacked
    bdec_d = din("bdec", (3, 2, D, 1))            # stacked (s|n) biases
    fcwT_d = din("fcwT", (T * D // 2, 512))       # fc1_w transposed
    fcb_d = din("fcb", (512,))
    means_s_d = din("means_s", (D // 2, NUM_M))
    meansT_s_d = din("meansT_s", (NUM_M, D // 2))
    means_n_d = din("means_n", (D // 2, NUM_N))
    meansT_n_d = din("meansT_n", (NUM_N, D // 2))

    s_hat_d = nc.dram_tensor("s_hat", [EPC, 512], F32, kind="ExternalOutput").ap()
    n_hat_d = nc.dram_tensor("n_hat", [EPC, 512], F32, kind="ExternalOutput").ap()
    idx_s_d = nc.dram_tensor("idx_s", [EPC, T], I32, kind="ExternalOutput").ap()
    idx_n_d = nc.dram_tensor("idx_n", [EPC, T], I32, kind="ExternalOutput").ap()

    id64_d = nc.inline_tensor(np.eye(D, dtype=np.float32), name="id64")
    id128_d = nc.inline_tensor(np.eye(128, dtype=np.float32), name="id128")

    with tile.TileContext(nc) as tc:
        with (
            tc.tile_pool(name="wpool", bufs=1) as wpool,
            tc.tile_pool(name="zpool", bufs=1) as zpool,
            tc.tile_pool(name="iopool", bufs=1) as iopool,
            tc.tile_pool(name="psum", bufs=1, space="PSUM") as pp,
        ):
            def wtile(shape, dtype, tag):
                return wpool.tile(list(shape), dtype, tag=tag, name=tag)

            # ------------- weights: DMA f32, build bf16 hi/lo ------------
            wstage = [wtile((128, 128), F32, f"wstage{i}") for i in range(3)]
            _ws = [0]

            def wst():
                t_ = wstage[_ws[0] % 3]
                _ws[0] += 1
                return t_

            def split_pair(shape, tag, src_ap):
                tf = wst()
                nc.sync.dma_start(tf[0:shape[0], 0:shape[1]], src_ap)
                hi = wtile(shape, BF16, tag + "h")
                lo = wtile(shape, BF16, tag + "l")
                nc.vector.tensor_copy(hi[:], tf[0:shape[0], 0:shape[1]])
                nc.vector.tensor_tensor(lo[:], tf[0:shape[0], 0:shape[1]],
                                        hi[:], op=SUB)
                return hi, lo

            w0c1h, w0c1l = split_pair((KW, 128), "w0c1", w0c1_d[:])

            w0c2h, w0c2l = [], []
            for p in range(5):
                h_, l_ = split_pair((128, D), f"w0c2{p}", w0c2_d[p])
                w0c2h.append(h_)
                w0c2l.append(l_)

            wench, wencl = {}, {}
            for i in range(3):
                for l in range(2):
                    for p in range(5):
                        h_, l_ = split_pair((128, D), f"we{i}{l}{p}",
                                            wenc_d[i, l, p])
                        wench[(i, l, p)] = h_
                        wencl[(i, l, p)] = l_

            wdecr = {}
            for i in range(3):
                for l in range(2):
                    for p in range(5):
                        tf = wst()
                        nc.sync.dma_start(tf[:, 0:D], wdec_d[i, l, p])
                        tr = wtile((128, D), F32R, f"wdr{i}{l}{p}")
                        nc.vector.tensor_copy(tr[:], tf[:, 0:D])
                        wdecr[(i, l, p)] = tr

            id64f = wtile((D, D), F32, "id64f")
            nc.sync.dma_start(id64f[:], id64_d.ap()[:])
            id64r = wtile((D, D), F32R, "id64r")
            nc.vector.tensor_copy(id64r[:], id64f[:])
            id64b = wtile((D, D), BF16, "id64b")
            nc.vector.tensor_copy(id64b[:], id64f[:])
            id128f = wtile((128, 128), F32, "id128f")
            nc.sync.dma_start(id128f[:], id128_d.ap()[:])

            def bias_tile(src_ap, n, tag):
                t_ = wtile((n, 1), F32, tag)
                nc.sync.dma_start(t_[:], src_ap)
                return t_

            b1e0 = bias_tile(b1e0_d, D, "b1e0")
            b2e0 = bias_tile(b2e0_d, D, "b2e0")
            bse0 = bias_tile(bse0_d, D, "bse0")
            b2bs = wtile((D, 1), F32, "b2bs")
            nc.vector.tensor_add(b2bs[:], b2e0[:], bse0[:])
            benc = {}
            bdec = {}
            for i in range(3):
                for l in range(2):
                    benc[(i, l)] = bias_tile(benc_d[i, l], D, f"benc{i}{l}")
                    bdec[(i, l)] = bias_tile(bdec_d[i, l], D, f"bdec{i}{l}")
            fcb = wtile((128, 4), F32, "fcb")
            nc.sync.dma_start(fcb[:], fcb_d.rearrange("(ob p) -> p ob", p=128))

            # ------------- VQ codebook prep ------------------------------
            vq = {}
            for cb, (mdim, mns_d, mnsT_d) in (
                ("s", (NUM_M, means_s_d, meansT_s_d)),
                ("n", (NUM_N, means_n_d, meansT_n_d)),
            ):
                mt = wtile((mdim, 32), F32, f"mt_{cb}")
                nc.sync.dma_start(mt[:], mnsT_d[:])
                mns = wtile((32, mdim), F32, f"mns_{cb}")
                nc.sync.dma_start(mns[:], mns_d[:])
                wg = wtile((33, mdim), F32, f"wg_{cb}")
                nc.vector.tensor_scalar_mul(wg[0:32, :], mns[:], 2.0)
                sq = wtile((mdim, 32), F32, f"sq_{cb}")
                nc.vector.tensor_mul(sq[:], mt[:], mt[:])
                m2 = wtile((mdim, 1), F32, f"m2_{cb}")
                nc.vector.reduce_sum(m2[:], sq[:], axis=mybir.AxisListType.X)
                nm2 = wtile((mdim, 1), F32, f"nm2_{cb}")
                nc.vector.tensor_scalar_mul(nm2[:], m2[:], -1.0)
                pt_ = pp.tile([1, mdim], F32, tag="psA0", name="prep_t")
                nc.tensor.transpose(pt_[:], nm2[:], id64f[0:mdim, 0:mdim])
                nc.scalar.copy(wg[32:33, :], pt_[:])
                wgr = wtile((33, mdim), F32R, f"wgr_{cb}")
                nc.vector.tensor_copy(wgr[:], wg[:])
                mTa = wtile((mdim, 33), F32, f"mTa_{cb}")
                nc.vector.tensor_copy(mTa[:, 0:32], mt[:])
                nc.vector.memset(mTa[:, 32:33], 1.0)
                mTar = wtile((mdim, 33), F32R, f"mTar_{cb}")
                nc.vector.tensor_copy(mTar[:], mTa[:])
                vq[cb] = dict(mdim=mdim, wg=wg, wgr=wgr, mTar=mTar)

            ones32f = wtile((33, 32), F32, "ones32f")
            nc.vector.memset(ones32f[0:1, :], 1.0)
            nc.vector.memset(ones32f[32:33, :], 1.0)
            ones32r = wtile((33, 32), F32R, "ones32r")
            nc.vector.tensor_copy(ones32r[0:1, :], ones32f[0:1, :])
            nc.vector.tensor_copy(ones32r[32:33, :], ones32f[32:33, :])

            # ------------- ring tiles ------------------------------------
            def ring(n, shape, dtype, tag):
                return [zpool.tile(list(shape), dtype, tag=f"{tag}{i}",
                                   name=f"{tag}{i}") for i in range(n)]

            ZW = T + 9            # 521
            x9_r = ring(3, (KW, T), F32, "x9")
            x9h_r = ring(2, (KW, T), BF16, "x9h")
            x9l_r = ring(2, (KW, T), BF16, "x9l")

            def zpair_ring(tag):
                return (ring(2, (128, ZW), BF16, tag + "h"),
                        ring(2, (128, ZW), BF16, tag + "l"))

            zA_r = zpair_ring("zA")     # enc0 conv2 input
            zB_r = zpair_ring("zB")     # enc0 out / block0 in
            z1_r = zpair_ring("z1")     # block0 out
            z2_r = zpair_ring("z2")     # block1 out
            zM_r = zpair_ring("zM")     # block mids
            ztf_r = ring(3, (D, ZW), F32, "ztf")      # enc evict staging
            h3s_r = ring(2, (33, T), F32, "h3s")
            h3n_r = ring(2, (33, T), F32, "h3n")
            h3sr_r = ring(2, (33, T), F32R, "h3sr")
            h3nr_r = ring(2, (33, T), F32R, "h3nr")
            e_s_r = ring(2, (NUM_M, T), F32R, "es")
            e_n_r = ring(2, (NUM_N, T), F32R, "en")
            qun_r = ring(1, (32, T), F32, "qun")
            rcpf_r = ring(2, (33, T), F32, "rcpf")
            rcprr_r = ring(2, (33, T), F32R, "rcprr")
            go_r = ring(1, (128, 4 * NUM_M + 4 * NUM_N), F32, "go")
            mx_r = ring(2, (128, 8), F32, "mx")
            ztmp_r = ring(3, (D, ZW), F32, "ztmp")    # q/dec evict staging
            zq_r = ring(2, (128, T + 8), F32R, "zq")  # dec conv inputs
            zdm_r = ring(2, (128, T + 8), F32R, "zdm")
            ztail_r = ring(2, (D, T), F32, "ztail")
            idx_r = {"s": ring(2, (128, 16, 4, 8), U32, "idxs"),
                     "n": ring(2, (128, 16, 4, 8), U32, "idxn")}

            ZT = [iopool.tile([128, 2, EPC, 32], BF16, tag=f"zt{tb}",
                              name=f"zt{tb}") for tb in range(4)]

            for z in ztf_r + ztmp_r:
                nc.gpsimd.memset(z[:, 0:4], 0.0)
                nc.gpsimd.memset(z[:, T + 4:ZW], 0.0)
            for h in h3s_r + h3n_r:
                nc.gpsimd.memset(h[32:33, :], 1.0)

            psA_r = [pp.tile([128, T], F32, tag=f"psA{i}", name=f"psA{i}")
                     for i in range(2)]
            psC_r = [pp.tile([D, T], F32, tag=f"psC{i}", name=f"psC{i}")
                     for i in range(4)]
            psGO_t = pp.tile([128, 4 * NUM_M + 4 * NUM_N], F32, tag="psGO",
                             name="psGO")
            psT_t = pp.tile([128, 256], F32, tag="psT", name="psT")
            psW_r = [pp.tile([128, 256], F32, tag=f"psC{i}", name=f"psW{i}")
                     for i in range(2)]
            _cnt = {}

            def nxt(name, lst):
                i = _cnt.get(name, 0)
                _cnt[name] = i + 1
                return lst[i % len(lst)]

            # ------------- stage helpers ---------------------------------
            def build_split(ps_in, bias, pair, f):
                """relu+bias evict -> f32 staging -> bf16 hi/lo doubled-shift."""
                ztf = nxt("ztf", ztf_r)
                nc.scalar.activation(ztf[:, 4:T + 4], ps_in, AF.Relu,
                                     bias=bias[:])
                zh, zl = pair[0][f % 2], pair[1][f % 2]
                nc.vector.tensor_copy(zh[0:D, 0:ZW], ztf[:, 0:ZW])
                nc.vector.tensor_copy(zh[D:128, 0:260], ztf[:, 1:261])
                nc.gpsimd.tensor_copy(zh[D:128, 260:ZW - 1], ztf[:, 261:ZW])
                nc.vector.tensor_tensor(zl[0:D, 0:ZW], ztf[:, 0:ZW],
                                        zh[0:D, 0:ZW], op=SUB)
                nc.gpsimd.tensor_copy(zl[D:128, 0:ZW - 1], zl[0:D, 1:ZW])

            def conv15(wh, wl, zh, zl, ps, first_start, last_stop):
                prods = ([(wh[p], zh, p) for p in range(5)]
                         + [(wl[p], zh, p) for p in range(5)]
                         + [(wh[p], zl, p) for p in range(5)])
                for k, (lhs, rhs, p) in enumerate(prods):
                    nc.tensor.matmul(
                        ps, lhs[:], rhs[:, 2 * p:2 * p + T],
                        start=(k == 0 and first_start),
                        stop=(k == 14 and last_stop),
                        skip_group_check=True)

            # --- encoder stages (element f = e+1 pipelined) ---
            def st_enc0_mm1(f):
                x9 = x9_r[f % 3]
                x9h, x9l = x9h_r[f % 2], x9l_r[f % 2]
                nc.vector.tensor_copy(x9h[:], x9[:])
                nc.vector.tensor_tensor(x9l[:], x9[:], x9h[:], op=SUB)
                psA = psA_r[f % 2]
                nc.tensor.matmul(psA[:], w0c1h[:], x9h[:], start=True,
                                 stop=False)
                nc.tensor.matmul(psA[:], w0c1l[:], x9h[:], start=False,
                                 stop=False, skip_group_check=True)
                nc.tensor.matmul(psA[:], w0c1h[:], x9l[:], start=False,
                                 stop=False, skip_group_check=True)
                build_split(psA[0:D, :], b1e0, zA_r, f)

            def st_enc0_conv2(f):
                psA = psA_r[f % 2]
                zh, zl = zA_r[0][f % 2], zA_r[1][f % 2]
                conv15(w0c2h, w0c2l, zh, zl, psA[D:128, :], False, True)
                build_split(psA[D:128, :], b2bs, zB_r, f)

            def _zin(f, i):
                src = (zB_r, z1_r, z2_r)[i]
                return src[0][f % 2], src[1][f % 2]

            def st_block_conv1(f, i):
                zh, zl = _zin(f, i)
                ps = nxt("psC", psC_r)
                conv15([wench[(i, 0, p)] for p in range(5)],
                       [wencl[(i, 0, p)] for p in range(5)],
                       zh, zl, ps[:], True, True)
                build_split(ps[:], benc[(i, 0)], zM_r, f)

            def st_block_conv2(f, i):
                zmh, zml = zM_r[0][f % 2], zM_r[1][f % 2]
                zih, zil = _zin(f, i)
                ps = nxt("psC", psC_r)
                conv15([wench[(i, 1, p)] for p in range(5)],
                       [wencl[(i, 1, p)] for p in range(5)],
                       zmh, zml, ps[:], True, False)
                nc.tensor.matmul(ps[:], id64b[:], zih[0:D, 4:T + 4],
                                 start=False, stop=False,
                                 skip_group_check=True)
                nc.tensor.matmul(ps[:], id64b[:], zil[0:D, 4:T + 4],
                                 start=False, stop=True,
                                 skip_group_check=True)
                if i < 2:
                    build_split(ps[:], benc[(i, 1)], (z1_r, z2_r)[i], f)
                else:
                    h3s, h3n = h3s_r[f % 2], h3n_r[f % 2]
                    nc.scalar.activation(h3s[0:32, :], ps[0:32, :], AF.Relu,
                                         bias=benc[(i, 1)][0:32, :])
                    nc.scalar.activation(h3n[0:32, :], ps[32:D, :], AF.Relu,
                                         bias=benc[(i, 1)][32:D, :])
                    nc.vector.tensor_copy(h3sr_r[f % 2][:], h3s[:])
                    nc.vector.tensor_copy(h3nr_r[f % 2][:], h3n[:])

            # --- VQ stages (element e) ---
            def st_vq_G(e):
                for cb in ("s", "n"):
                    v = vq[cb]
                    mdim = v["mdim"]
                    h3cbr = (h3sr_r if cb == "s" else h3nr_r)[e % 2]
                    psG = nxt("psC", psC_r)
                    nc.tensor.matmul(psG[0:mdim, :], v["wgr"][:], h3cbr[:])
                    ecb = (e_s_r if cb == "s" else e_n_r)[e % 2]
                    nc.scalar.activation(ecb[:], psG[0:mdim, :], AF.Exp,
                                         scale=SCALE)

            def st_vq_GO(e):
                go = go_r[0]
                for cb in ("s", "n"):
                    v = vq[cb]
                    mdim = v["mdim"]
                    h3cb = (h3s_r if cb == "s" else h3n_r)[e % 2]
                    goff = 0 if cb == "s" else 4 * NUM_M
                    for j in range(4):
                        nc.tensor.matmul(
                            psGO_t[:, goff + j * mdim:goff + (j + 1) * mdim],
                            h3cb[:, 128 * j:128 * (j + 1)], v["wg"][:])
                    nc.scalar.copy(go[:, goff:goff + 4 * mdim],
                                   psGO_t[:, goff:goff + 4 * mdim])

            def st_vq_U(e):
                for cb in ("s", "n"):
                    v = vq[cb]
                    ecb = (e_s_r if cb == "s" else e_n_r)[e % 2]
                    psU = nxt("psC", psC_r)
                    nc.tensor.matmul(psU[0:33, :], v["mTar"][:], ecb[:])
                    ci_ = 0 if cb == "s" else 32
                    rcp = rcpf_r[e % 2][ci_:ci_ + 1, :]
                    rcpr = rcprr_r[e % 2][ci_:ci_ + 1, :]
                    nc.vector.reciprocal(rcp, psU[32:33, :])
                    nc.vector.tensor_copy(rcpr, rcp)
                    v["psU_live"] = psU

            def st_vq_bc(e):
                for cb in ("s", "n"):
                    v = vq[cb]
                    ci_ = 0 if cb == "s" else 32
                    rcpr = rcprr_r[e % 2][ci_:ci_ + 1, :]
                    psR = nxt("psC", psC_r)
                    nc.tensor.matmul(psR[0:32, :],
                                     ones32r[ci_:ci_ + 1, :], rcpr)
                    v["psR_live"] = psR

            def st_vq_tail(e):
                em = e % 16
                ztq = nxt("ztmp", ztmp_r)
                go = go_r[0]
                for ci, cb in enumerate(("s", "n")):
                    v = vq[cb]
                    mdim = v["mdim"]
                    qun = qun_r[0]
                    nc.scalar.copy(qun[:], v["psU_live"][0:32, :])
                    nc.vector.tensor_mul(ztq[32 * ci:32 * ci + 32, 4:T + 4],
                                         qun[:], v["psR_live"][0:32, :])
                    goff = 0 if cb == "s" else 4 * NUM_M
                    mx = mx_r[e % 2]
                    for j in range(4):
                        nc.vector.max(mx[:], go[:, goff + j * mdim:
                                                goff + (j + 1) * mdim])
                        nc.vector.max_index(
                            idx_r[cb][(e // 16) % 2][:, em, j, :], mx[:],
                            go[:, goff + j * mdim:goff + (j + 1) * mdim])
                zq = zq_r[e % 2]
                nc.vector.tensor_copy(zq[0:D, 0:T + 8], ztq[:, 0:T + 8])
                nc.vector.tensor_copy(zq[D:128, 0:260], ztq[:, 1:261])
                nc.gpsimd.tensor_copy(zq[D:128, 260:T + 8], ztq[:, 261:T + 9])
                if em == 15:
                    e0 = e - 15
                    for cb, dram in (("s", idx_s_d), ("n", idx_n_d)):
                        nc.sync.dma_start(
                            dram[e0:e0 + 16, :].rearrange(
                                "e (j p) -> p e j", p=128),
                            idx_r[cb][(e // 16) % 2][:, :, :, 0].bitcast(I32))

            # --- decoder stages (element e) ---
            def _zdec(e, i):
                return (zq_r[e % 2], zq_r[(e + 1) % 2], zq_r[e % 2])[i]

            def st_dec_conv1(e, i):
                z_dec = _zdec(e, i)
                ps = nxt("psC", psC_r)
                for p in range(5):
                    nc.tensor.matmul(ps[:], wdecr[(i, 0, p)][:],
                                     z_dec[:, 2 * p:2 * p + T],
                                     start=(p == 0), stop=(p == 4))
                ztd = nxt("ztmp", ztmp_r)
                nc.scalar.activation(ztd[:, 4:T + 4], ps[:], AF.Relu,
                                     bias=bdec[(i, 0)][:])
                zdm = zdm_r[e % 2]
                nc.vector.tensor_copy(zdm[0:D, 0:T + 8], ztd[:, 0:T + 8])
                nc.gpsimd.tensor_copy(zdm[D:128, 0:T + 8], ztd[:, 1:T + 9])

            def st_dec_conv2(e, i):
                z_dec = _zdec(e, i)
                zdm = zdm_r[e % 2]
                ps = nxt("psC", psC_r)
                nc.tensor.matmul(ps[:], id64r[:], z_dec[0:D, 4:T + 4],
                                 start=True, stop=False)
                for p in range(5):
                    nc.tensor.matmul(ps[:], wdecr[(i, 1, p)][:],
                                     zdm[:, 2 * p:2 * p + T],
                                     start=False, stop=(p == 4),
                                     skip_group_check=True)
                if i < 2:
                    ztd2 = nxt("ztmp", ztmp_r)
                    nc.scalar.activation(ztd2[:, 4:T + 4], ps[:], AF.Relu,
                                         bias=bdec[(i, 1)][:])
                    zn = _zdec(e, i + 1)
                    nc.vector.tensor_copy(zn[0:D, 0:T + 8], ztd2[:, 0:T + 8])
                    nc.gpsimd.tensor_copy(zn[D:128, 0:T + 8], ztd2[:, 1:T + 9])
                else:
                    ztl = ztail_r[e % 2]
                    nc.scalar.activation(ztl[:], ps[:], AF.Relu,
                                         bias=bdec[(i, 1)][:])

            def st_tail(e):
                ztl = ztail_r[e % 2]
                for tb in range(4):
                    nc.tensor.transpose(psT_t[:, tb * D:(tb + 1) * D],
                                        ztl[:, 128 * tb:128 * (tb + 1)],
                                        id64f[:])
                for tb in range(4):
                    nc.vector.tensor_copy(
                        ZT[tb][:, :, e, :],
                        psT_t[:, tb * D:(tb + 1) * D].rearrange(
                            "p (st c) -> p st c", st=2))

            # ------------- software-pipelined element loop ---------------
            def dma_x(f):
                nc.sync.dma_start(x9_r[f % 3][:], xcol_d[f])

            dma_x(0)
            if N_ELEM > 1:
                dma_x(1)
            st_enc0_mm1(0)
            st_enc0_conv2(0)
            for i in range(3):
                st_block_conv1(0, i)
                st_block_conv2(0, i)

            for e in range(N_ELEM):
                nxt_e = e + 1 if e + 1 < N_ELEM else None
                if nxt_e is not None and nxt_e + 1 < N_ELEM:
                    dma_x(nxt_e + 1)
                st_vq_G(e)
                if nxt_e is not None:
                    st_enc0_mm1(nxt_e)
                st_vq_GO(e)
                st_vq_U(e)
                if nxt_e is not None:
                    st_enc0_conv2(nxt_e)
                st_vq_bc(e)
                st_vq_tail(e)
                for i in range(3):
                    if nxt_e is not None:
                        st_block_conv1(nxt_e, i)
                    st_dec_conv1(e, i)
                    if nxt_e is not None:
                        st_block_conv2(nxt_e, i)
                    st_dec_conv2(e, i)
                st_tail(e)

            # flush partial idx ring (only when N_ELEM % 16 != 0)
            rem = N_ELEM % 16
            if rem:
                e0 = N_ELEM - rem
                for cb, dram in (("s", idx_s_d), ("n", idx_n_d)):
                    nc.sync.dma_start(
                        dram[e0:e0 + rem, :].rearrange(
                            "e (j p) -> p e j", p=128),
                        idx_r[cb][(e0 // 16) % 2][:, 0:rem, :, 0].bitcast(I32))

            # ------------- FC phase (bf16) -------------------------------
            fcw_f = [zpool.tile([128, 512], F32, tag=f"fcwf{i}",
                                name=f"fcwf{i}") for i in range(4)]
            fcw_b = [zpool.tile([128, 512], BF16, tag=f"fcwb{i}",
                                name=f"fcwb{i}") for i in range(4)]
            sf_r = [zpool.tile([128, 256], F32, tag=f"sf{i}", name=f"sf{i}")
                    for i in range(2)]
            sft_r = [zpool.tile([128, 128], F32, tag=f"sft{i}", name=f"sft{i}")
                     for i in range(4)]
            psF4 = psW_r + [pp.tile([128, 256], F32, tag=f"psC{i + 2}",
                                    name=f"psW{i + 2}") for i in range(2)]
            for c in range(128):
                wf = fcw_f[c % 4]
                wb = fcw_b[c % 4]
                nc.sync.dma_start(wf[:], fcwT_d[c * 128:(c + 1) * 128, :])
                nc.vector.tensor_copy(wb[:], wf[:])
                for ob in range(4):
                    nc.tensor.matmul(
                        psF4[ob][:], wb[:, ob * 128:(ob + 1) * 128],
                        ZT[c % 4][:, :, :, c // 4],
                        start=(c == 0), stop=(c == 127))
            for ob in range(4):
                sf = sf_r[ob % 2]
                nc.scalar.activation(sf[:], psF4[ob][:], AF.Tanh,
                                     bias=fcb[:, ob:ob + 1])
                for st, dram in ((0, s_hat_d), (1, n_hat_d)):
                    psT2 = psT_t if st == 0 else psGO_t
                    nc.tensor.transpose(psT2[:, 0:128],
                                        sf[:, st * 128:(st + 1) * 128],
                                        id128f[:])
                    sft = sft_r[ob % 2 * 2 + st]
                    nc.scalar.copy(sft[:], psT2[:, 0:128])
                    nc.sync.dma_start(
                        dram[0:EPC, ob * 128:(ob + 1) * 128], sft[:])

    nc.compile()
    return nc


def _prep_host(inputs):
    """Host-side layout transforms (pad / im2col / transpose / stack only)."""
    f = np.float32
    x = np.asarray(inputs["x"], f)
    xpad = np.pad(x, ((0, 0), (4, 4)))
    xcol = np.ascontiguousarray(
        np.lib.stride_tricks.sliding_window_view(xpad, T, axis=1))
    assert xcol.shape == (B, KW, T), xcol.shape  # xcol[b,k,t] = xpad[b,k+t]

    w1 = np.asarray(inputs["enc0_w1"], f)     # (64,1,9)
    ws = np.asarray(inputs["enc0_ws"], f)     # (64,1,1)
    w0c1 = np.zeros((KW, 128), f)
    w0c1[:, 0:D] = w1[:, 0, :].T
    w0c1[4, D:128] = ws[:, 0, 0]

    def pairs(w):  # (Cout,Cin,9) -> (5, 2*Cin, Cout)
        co, ci, _ = w.shape
        out = np.zeros((5, 2 * ci, co), f)
        for p in range(5):
            for j in range(2):
                kk = 2 * p + j
                if kk < KW:
                    out[p, j * ci:(j + 1) * ci, :] = w[:, :, kk].T
        return out

    w0c2 = pairs(np.asarray(inputs["enc0_w2"], f))
    wenc = np.stack([np.stack([pairs(np.asarray(inputs["enc_w"][i, l], f))
                               for l in range(2)]) for i in range(3)])

    def dec_pairs(w):  # (32,32,9) -> (5, 128, 64) stream-stacked block-diag
        out = np.zeros((5, 128, D), f)
        pw = pairs(w)  # (5, 64, 32)
        for p in range(5):
            for j in range(2):
                blk = pw[p, j * 32:(j + 1) * 32, :]  # (ci, co)
                for st in range(2):
                    out[p, j * 64 + st * 32:j * 64 + (st + 1) * 32,
                        st * 32:(st + 1) * 32] = blk
        return out

    wdec = np.stack([np.stack([dec_pairs(np.asarray(inputs["dec_w"][i, l], f))
                               for l in range(2)]) for i in range(3)])
    bdec = np.stack([np.stack([np.tile(np.asarray(inputs["dec_b"][i, l], f), 2)
                               for l in range(2)]) for i in range(3)])

    fcwT = np.ascontiguousarray(np.asarray(inputs["fc1_w"], f).T)
    means_s = np.asarray(inputs["means_s"], f)
    means_n = np.asarray(inputs["means_n"], f)

    common = dict(
        w0c1=w0c1, b1e0=np.asarray(inputs["enc0_b1"], f)[:, None], w0c2=w0c2,
        b2e0=np.asarray(inputs["enc0_b2"], f)[:, None],
        bse0=np.asarray(inputs["enc0_bs"], f)[:, None],
        wenc=wenc, benc=np.asarray(inputs["enc_b"], f)[..., None],
        wdec=wdec, bdec=bdec[..., None], fcwT=fcwT,
        fcb=np.asarray(inputs["fc1_b"], f),
        means_s=means_s, meansT_s=np.ascontiguousarray(means_s.T),
        means_n=means_n, meansT_n=np.ascontiguousarray(means_n.T),
    )
    in_maps = []
    for c in range(NCORES):
        m = dict(common)
        m["xcol"] = np.ascontiguousarray(xcol[c * EPC:(c + 1) * EPC])
        in_maps.append(m)
    return in_maps


TRACE = False
TRACE_DIR = None
LAST_RESULTS = None


def kernel(**inputs):
    global _COMPILED, LAST_RESULTS
    from concourse.bass_utils import run_bass_kernel_spmd

    if _COMPILED is None:
        _COMPILED = _build()
    in_maps = _prep_host(inputs)
    kw = {}
    if TRACE:
        kw = dict(trace=True, tmpdir=TRACE_DIR)
    res = run_bass_kernel_spmd(_COMPILED, in_maps, core_ids=list(range(NCORES)),
                               **kw)
    LAST_RESULTS = res
    s_hat = np.concatenate([res.results[c]["s_hat"] for c in range(NCORES)], 0)
    n_hat = np.concatenate([res.results[c]["n_hat"] for c in range(NCORES)], 0)
    idx_s = np.concatenate([res.results[c]["idx_s"] for c in range(NCORES)], 0)
    idx_n = np.concatenate([res.results[c]["idx_n"] for c in range(NCORES)], 0)
    return s_hat, n_hat, idx_s.astype(np.int32), idx_n.astype(np.int32)
